# revision 24
# baseline (speedup 1.0000x reference)
"""Multi-head attention (B=4, N=4096, E=256, H=4) + output projection on
8 Trainium2 NeuronCores.

Sharding: data-parallel over (batch, query-half) -> 8 shards. Each core
computes full 4-head attention for one batch's 2048 queries against that
batch's full K/V, applies the output projection (+bias), and writes its
[2048, 256] f32 slice. No collectives needed; the host concatenates.

Per-core kernel (flash-attention style, S^T layout, bf16 compute):
  for each 512-query block, head-pair (2 heads), 128-key chunk:
    S^T_h0,S^T_h1 = row-tiled TensorE matmuls (K=64 contraction) -> PSUM
    P^T = exp(S^T/8)  -- one ScalarE ACTIVATE [128,1024] -> SBUF bf16
    out^T += col-tiled (V_h0|V_h1) matmuls     (PSUM accumulate)
    rowsums += col-tiled (ones|ones) matmuls   (64-replica trick)
  ctxT = out^T * reciprocal(rowsums)           (VectorE, off critical path)
  y = ctxT.T @ W_out.T + b_out                 (TensorE, K=1 bias matmul)

Host-side prep is layout only: transpose/cast shards to bf16 in the
layouts the TensorE wants (lhsT convention), plus the V swizzle.
"""

import os
import tempfile
from contextlib import ExitStack

import ml_dtypes
import numpy as np

import concourse.bass as bass
import concourse.tile as tile
from concourse import bacc, mybir
from concourse.bass_utils import run_bass_kernel_spmd

BF16 = mybir.dt.bfloat16
F32 = mybir.dt.float32

B, N, E = 4, 4096, 256
H, D = 4, 64
QLEN = N // 2
N_CORES = 8

LAST_EXEC_TIME_NS = None
_NC_CACHE = {}


def _build(qlen=QLEN, seq=N, n_cores=N_CORES):
    n_kc = seq // 128
    n_q = qlen // 512
    QB = 512

    nc = bacc.Bacc("TRN2", target_bir_lowering=False, debug=False, num_devices=n_cores)

    qt_d = nc.dram_tensor("qt", [2, 128, qlen], BF16, kind="ExternalInput").ap()
    kt_d = nc.dram_tensor("kt", [2, 128, seq], BF16, kind="ExternalInput").ap()
    v_d = nc.dram_tensor("v", [128, n_kc * 256], BF16, kind="ExternalInput").ap()
    wt_d = nc.dram_tensor("wt", [2, 128, 256], BF16, kind="ExternalInput").ap()
    bias_d = nc.dram_tensor("bias", [1, 256], BF16, kind="ExternalInput").ap()
    y_d = nc.dram_tensor("y", [qlen, 256], F32, kind="ExternalOutput").ap()

    with tile.TileContext(nc) as tc, ExitStack() as ctx:
        const = ctx.enter_context(tc.tile_pool(name="const", bufs=1))
        ep_pool = ctx.enter_context(tc.tile_pool(name="ep", bufs=3))
        y_pool = ctx.enter_context(tc.tile_pool(name="ysb", bufs=2))
        st_pool = ctx.enter_context(tc.tile_pool(name="st", bufs=3, space="PSUM"))
        part_pool = ctx.enter_context(tc.tile_pool(name="part", bufs=1, space="PSUM"))
        pt_pool = ctx.enter_context(tc.tile_pool(name="pt", bufs=8))

        qt_sb = [
            const.tile([128, qlen], BF16, tag=f"qt{p}", name=f"qt_sb{p}")
            for p in range(2)
        ]
        kt_sb = [
            const.tile([128, seq], BF16, tag=f"kt{p}", name=f"kt_sb{p}")
            for p in range(2)
        ]
        v_sb = const.tile([128, n_kc * 256], BF16, tag="v")
        wt_sb = [
            const.tile([128, 256], BF16, tag=f"wt{p}", name=f"wt_sb{p}")
            for p in range(2)
        ]
        bias_sb = const.tile([1, 256], BF16, tag="bias")
        ones64 = const.tile([128, 64], BF16, tag="ones64")
        ones1 = const.tile([1, 128], BF16, tag="ones1")
        zbias = const.tile([128, 1], F32, tag="zbias")
        ctx_sb = [
            const.tile([128, qlen], BF16, tag=f"ctx{p}", name=f"ctx_sb{p}")
            for p in range(2)
        ]

        # DMA issue costs ~650ns/inst on a HWDGE queue: split the loads
        # across BOTH rings (SP + ACT engines) and order by first use.
        # SP ring: pair01 q/k (first chunks first); ACT ring: v, pair23, w.
        qsp = min(512, qlen)
        ksp = min(1024, seq)
        vsp = min(2048, n_kc * 256)
        ksp0 = min(512, seq)
        nc.sync.dma_start(bias_sb[:], bias_d)
        nc.sync.dma_start(qt_sb[0][:, 0:qsp], qt_d[0][:, 0:qsp])
        nc.sync.dma_start(kt_sb[0][:, 0:ksp0], kt_d[0][:, 0:ksp0])
        if ksp0 < ksp:
            nc.sync.dma_start(kt_sb[0][:, ksp0:ksp], kt_d[0][:, ksp0:ksp])
        nc.sync.dma_start(v_sb[:, 0:vsp], v_d[:, 0:vsp])
        if qsp < qlen:
            nc.sync.dma_start(qt_sb[0][:, qsp:], qt_d[0][:, qsp:])
        if ksp < seq:
            nc.sync.dma_start(kt_sb[0][:, ksp:], kt_d[0][:, ksp:])
        if vsp < n_kc * 256:
            nc.sync.dma_start(v_sb[:, vsp:], v_d[:, vsp:])
        nc.sync.dma_start(qt_sb[1][:], qt_d[1])
        nc.sync.dma_start(kt_sb[1][:], kt_d[1])
        for p in range(2):
            nc.sync.dma_start(wt_sb[p][:], wt_d[p])
        bias_bc = const.tile([128, 256], F32, tag="bias_bc")
        # HAM warmup: ~6us of dependency-free matmuls so the PE clock-gate
        # is at 2.4 GHz when the first real QK lands (runs during the DMA
        # lead-in; uses the part slot, whose first real use is much later)
        dummy = const.tile([128, 512], BF16, tag="dummy")
        nc.vector.memset(dummy[:], 0.0)
        nc.vector.memset(ones64[:], 1.0)
        nc.vector.memset(ones1[:], 1.0)
        nc.vector.memset(zbias[:], 0.0)
        warm_ps = part_pool.tile([128, 2 * QB], F32, tag="part", name="warm_ps")
        # b_out broadcast to 128 partitions (projection epilogue adds);
        # computed during the DMA lead-in in the warmup's PSUM slot
        nc.tensor.matmul(
            warm_ps[:, QB : QB + 256], ones1[:, 0:128], bias_sb[:], start=True, stop=True
        )
        nc.vector.tensor_copy(bias_bc[:], warm_ps[:, QB : QB + 256])
        for _ in range(9):
            nc.tensor.matmul(
                warm_ps[0:64, 0:QB], ones64[:, :], dummy[:, :], start=True, stop=True
            )

        def emit_qk_exp(pair, qs, kc):
            ks = slice(kc * 128, (kc + 1) * 128)
            st = st_pool.tile([128, 2 * QB], F32, name="st")
            nc.tensor.matmul(
                st[:, 0:QB],
                kt_sb[pair][0:64, ks],
                qt_sb[pair][0:64, qs],
                start=True,
                stop=True,
                tile_position=(0, 0),
            )
            nc.tensor.matmul(
                st[:, QB : 2 * QB],
                kt_sb[pair][64:128, ks],
                qt_sb[pair][64:128, qs],
                start=True,
                stop=True,
                tile_position=(64, 0),
            )
            pt = pt_pool.tile([128, 2 * QB], BF16, name="pt")
            nc.scalar.activation(
                pt[:],
                st[:],
                mybir.ActivationFunctionType.Exp,
                bias=zbias[:, 0:1],
                scale=0.125,
            )
            return pt

        def emit_av_sums(pair, out_part, sum_part, pt, kc):
            first = kc == 0
            last = kc == n_kc - 1
            vcol = kc * 256 + pair * 128
            nc.tensor.matmul(
                out_part[0:64],
                v_sb[:, vcol : vcol + 64],
                pt[:, 0:QB],
                start=first,
                stop=last,
                tile_position=(0, 0),
                skip_group_check=True,
            )
            nc.tensor.matmul(
                out_part[64:128],
                v_sb[:, vcol + 64 : vcol + 128],
                pt[:, QB : 2 * QB],
                start=first,
                stop=last,
                tile_position=(0, 64),
                skip_group_check=True,
            )
            nc.tensor.matmul(
                sum_part[0:64],
                ones64[:, :],
                pt[:, 0:QB],
                start=first,
                stop=last,
                tile_position=(0, 0),
                skip_group_check=True,
            )
            nc.tensor.matmul(
                sum_part[64:128],
                ones64[:, :],
                pt[:, QB : 2 * QB],
                start=first,
                stop=last,
                tile_position=(0, 64),
                skip_group_check=True,
            )

        # one flat chunk stream, software-pipelined globally: QK/exp of
        # chunk i+2 is emitted before AV/sums of chunk i, across (Q, pair)
        # boundaries, so the ScalarE exp stream never stalls
        chunks = [
            (qi, pair, kc)
            for qi in range(n_q)
            for pair in range(2)
            for kc in range(n_kc)
        ]
        nch = len(chunks)

        def qk(i):
            qi, pair, kc = chunks[i]
            return emit_qk_exp(pair, slice(qi * QB, (qi + 1) * QB), kc)

        y_t = y_d.rearrange("(t p) e -> p t e", p=128)
        n_qt = qlen // 128
        y_state = {}

        def emit_proj(qt_i):
            ps = slice(qt_i * 128, (qt_i + 1) * 128)
            y_ps = st_pool.tile([128, 256], F32, tag="st", name="y_ps")
            nc.tensor.matmul(
                y_ps[:], ctx_sb[0][:, ps], wt_sb[0][:], start=True, stop=False
            )
            nc.tensor.matmul(
                y_ps[:], ctx_sb[1][:, ps], wt_sb[1][:], start=False, stop=True
            )
            if qt_i % 2 == 0:
                y_state["sb"] = y_pool.tile([128, 512], F32, bufs=4, name="y_sb2")
            half = slice((qt_i % 2) * 256, (qt_i % 2) * 256 + 256)
            nc.vector.tensor_add(y_state["sb"][:, half], y_ps[:], bias_bc[:])
            if qt_i % 2 == 1 or qt_i == n_qt - 1:
                lo = (qt_i // 2) * 2
                hi = min(lo + 2, n_qt)
                eng = nc.sync if (qt_i // 2) % 2 == 0 else nc.scalar
                eng.dma_start(y_t[:, lo:hi, :], y_state["sb"][:, 0 : (hi - lo) * 256])

        proj_done = 0
        pts = {}
        pts[0] = qk(0)
        if nch > 1:
            pts[1] = qk(1)
        part = None
        for i, (qi, pair, kc) in enumerate(chunks):
            if i + 2 < nch:
                pts[i + 2] = qk(i + 2)
            if kc == 0:
                part = part_pool.tile([128, 2 * QB], F32, tag="part", name="part")
            emit_av_sums(pair, part[:, 0:QB], part[:, QB : 2 * QB], pts.pop(i), kc)
            # previous Q-block's projection rides in this block's PE slack
            if qi >= 1 and pair == 0 and kc in (14, 17, 20, 23):
                qt_i = (qi - 1) * 4 + (kc - 14) // 3
                if qt_i < n_qt:
                    emit_proj(qt_i)
                    proj_done = qt_i + 1
            if kc == n_kc - 1:
                # stage partials to SBUF (releases the PSUM banks), then
                # normalize off the critical path. On the very last pair
                # ScalarE is idle: split the copy across ACT+DVE to shorten
                # the tail's serial chain.
                qs = slice(qi * QB, (qi + 1) * QB)
                stage = ep_pool.tile([128, 2 * QB], F32, tag="stage")
                last_ep = qi == n_q - 1 and pair == 1
                if last_ep:
                    nc.scalar.copy(stage[:, 0:QB], part[:, 0:QB])
                    nc.vector.tensor_copy(stage[:, QB : 2 * QB], part[:, QB : 2 * QB])
                else:
                    nc.vector.tensor_copy(stage[:], part[:])
                recip = ep_pool.tile([128, QB], F32, tag="recip")
                nc.vector.reciprocal_approx_fast(recip[:], stage[:, QB : 2 * QB])
                nc.vector.tensor_mul(ctx_sb[pair][:, qs], stage[:, 0:QB], recip[:])

        # remaining projection tiles (last Q-block)
        for qt_i in range(proj_done, n_qt):
            emit_proj(qt_i)

    nc.compile()
    return nc


# revision 25
# speedup vs baseline: 1.0002x; 1.0002x over previous
"""Multi-head attention (B=4, N=4096, E=256, H=4) + output projection on
8 Trainium2 NeuronCores.

Sharding: data-parallel over (batch, query-half) -> 8 shards. Each core
computes full 4-head attention for one batch's 2048 queries against that
batch's full K/V, applies the output projection (+bias), and writes its
[2048, 256] f32 slice. No collectives needed; the host concatenates.

Per-core kernel (flash-attention style, S^T layout, bf16 compute):
  for each 512-query block, head-pair (2 heads), 128-key chunk:
    S^T_h0,S^T_h1 = row-tiled TensorE matmuls (K=64 contraction) -> PSUM
    P^T = exp(S^T/8)  -- one ScalarE ACTIVATE [128,1024] -> SBUF bf16
    out^T += col-tiled (V_h0|V_h1) matmuls     (PSUM accumulate)
    rowsums += col-tiled (ones|ones) matmuls   (64-replica trick)
  ctxT = out^T * reciprocal(rowsums)           (VectorE, off critical path)
  y = ctxT.T @ W_out.T + b_out                 (TensorE, K=1 bias matmul)

Host-side prep is layout only: transpose/cast shards to bf16 in the
layouts the TensorE wants (lhsT convention), plus the V swizzle.
"""

import os
import tempfile
from contextlib import ExitStack

import ml_dtypes
import numpy as np

import concourse.bass as bass
import concourse.tile as tile
from concourse import bacc, mybir
from concourse.bass_utils import run_bass_kernel_spmd

BF16 = mybir.dt.bfloat16
F32 = mybir.dt.float32

B, N, E = 4, 4096, 256
H, D = 4, 64
QLEN = N // 2
N_CORES = 8

LAST_EXEC_TIME_NS = None
_NC_CACHE = {}


def _build(qlen=QLEN, seq=N, n_cores=N_CORES):
    n_kc = seq // 128
    n_q = qlen // 512
    QB = 512

    nc = bacc.Bacc("TRN2", target_bir_lowering=False, debug=False, num_devices=n_cores)

    qt_d = nc.dram_tensor("qt", [2, 128, qlen], BF16, kind="ExternalInput").ap()
    kt_d = nc.dram_tensor("kt", [2, 128, seq], BF16, kind="ExternalInput").ap()
    v_d = nc.dram_tensor("v", [128, n_kc * 256], BF16, kind="ExternalInput").ap()
    wt_d = nc.dram_tensor("wt", [2, 128, 256], BF16, kind="ExternalInput").ap()
    bias_d = nc.dram_tensor("bias", [1, 256], BF16, kind="ExternalInput").ap()
    y_d = nc.dram_tensor("y", [qlen, 256], F32, kind="ExternalOutput").ap()

    with tile.TileContext(nc) as tc, ExitStack() as ctx:
        const = ctx.enter_context(tc.tile_pool(name="const", bufs=1))
        ep_pool = ctx.enter_context(tc.tile_pool(name="ep", bufs=2))
        y_pool = ctx.enter_context(tc.tile_pool(name="ysb", bufs=2))
        st_pool = ctx.enter_context(tc.tile_pool(name="st", bufs=3, space="PSUM"))
        part_pool = ctx.enter_context(tc.tile_pool(name="part", bufs=1, space="PSUM"))
        pt_pool = ctx.enter_context(tc.tile_pool(name="pt", bufs=8))

        qt_sb = [
            const.tile([128, qlen], BF16, tag=f"qt{p}", name=f"qt_sb{p}")
            for p in range(2)
        ]
        kt_sb = [
            const.tile([128, seq], BF16, tag=f"kt{p}", name=f"kt_sb{p}")
            for p in range(2)
        ]
        v_sb = const.tile([128, n_kc * 256], BF16, tag="v")
        wt_sb = [
            const.tile([128, 256], BF16, tag=f"wt{p}", name=f"wt_sb{p}")
            for p in range(2)
        ]
        bias_sb = const.tile([1, 256], BF16, tag="bias")
        ones64 = const.tile([128, 64], BF16, tag="ones64")
        ones1 = const.tile([1, 128], BF16, tag="ones1")
        zbias = const.tile([128, 1], F32, tag="zbias")
        ctx_sb = [
            const.tile([128, qlen], BF16, tag=f"ctx{p}", name=f"ctx_sb{p}")
            for p in range(2)
        ]

        # DMA issue costs ~650ns/inst on a HWDGE queue: split the loads
        # across BOTH rings (SP + ACT engines) and order by first use.
        # SP ring: pair01 q/k (first chunks first); ACT ring: v, pair23, w.
        qsp = min(512, qlen)
        ksp = min(1024, seq)
        vsp = min(2048, n_kc * 256)
        ksp0 = min(512, seq)
        nc.sync.dma_start(bias_sb[:], bias_d)
        nc.sync.dma_start(qt_sb[0][:, 0:qsp], qt_d[0][:, 0:qsp])
        nc.sync.dma_start(kt_sb[0][:, 0:ksp0], kt_d[0][:, 0:ksp0])
        if ksp0 < ksp:
            nc.sync.dma_start(kt_sb[0][:, ksp0:ksp], kt_d[0][:, ksp0:ksp])
        nc.sync.dma_start(v_sb[:, 0:vsp], v_d[:, 0:vsp])
        if qsp < qlen:
            nc.sync.dma_start(qt_sb[0][:, qsp:], qt_d[0][:, qsp:])
        if ksp < seq:
            nc.sync.dma_start(kt_sb[0][:, ksp:], kt_d[0][:, ksp:])
        if vsp < n_kc * 256:
            nc.sync.dma_start(v_sb[:, vsp:], v_d[:, vsp:])
        nc.sync.dma_start(qt_sb[1][:], qt_d[1])
        nc.sync.dma_start(kt_sb[1][:], kt_d[1])
        for p in range(2):
            nc.sync.dma_start(wt_sb[p][:], wt_d[p])
        bias_bc = const.tile([128, 256], F32, tag="bias_bc")
        # HAM warmup: ~6us of dependency-free matmuls so the PE clock-gate
        # is at 2.4 GHz when the first real QK lands (runs during the DMA
        # lead-in; uses the part slot, whose first real use is much later)
        dummy = const.tile([128, 512], BF16, tag="dummy")
        nc.vector.memset(dummy[:], 0.0)
        nc.vector.memset(ones64[:], 1.0)
        nc.vector.memset(ones1[:], 1.0)
        nc.vector.memset(zbias[:], 0.0)
        warm_ps = part_pool.tile([128, 2 * QB], F32, tag="part", name="warm_ps")
        # b_out broadcast to 128 partitions (projection epilogue adds);
        # computed during the DMA lead-in in the warmup's PSUM slot
        nc.tensor.matmul(
            warm_ps[:, QB : QB + 256], ones1[:, 0:128], bias_sb[:], start=True, stop=True
        )
        nc.vector.tensor_copy(bias_bc[:], warm_ps[:, QB : QB + 256])
        for _ in range(9):
            nc.tensor.matmul(
                warm_ps[0:64, 0:QB], ones64[:, :], dummy[:, :], start=True, stop=True
            )

        def emit_qk_exp(pair, qs, kc):
            ks = slice(kc * 128, (kc + 1) * 128)
            st = st_pool.tile([128, 2 * QB], F32, name="st")
            nc.tensor.matmul(
                st[:, 0:QB],
                kt_sb[pair][0:64, ks],
                qt_sb[pair][0:64, qs],
                start=True,
                stop=True,
                tile_position=(0, 0),
            )
            nc.tensor.matmul(
                st[:, QB : 2 * QB],
                kt_sb[pair][64:128, ks],
                qt_sb[pair][64:128, qs],
                start=True,
                stop=True,
                tile_position=(64, 0),
            )
            pt = pt_pool.tile([128, 2 * QB], BF16, name="pt")
            nc.scalar.activation(
                pt[:],
                st[:],
                mybir.ActivationFunctionType.Exp,
                bias=zbias[:, 0:1],
                scale=0.125,
            )
            return pt

        def emit_av_sums(pair, out_part, sum_part, pt, kc):
            first = kc == 0
            last = kc == n_kc - 1
            vcol = kc * 256 + pair * 128
            nc.tensor.matmul(
                out_part[0:64],
                v_sb[:, vcol : vcol + 64],
                pt[:, 0:QB],
                start=first,
                stop=last,
                tile_position=(0, 0),
                skip_group_check=True,
            )
            nc.tensor.matmul(
                out_part[64:128],
                v_sb[:, vcol + 64 : vcol + 128],
                pt[:, QB : 2 * QB],
                start=first,
                stop=last,
                tile_position=(0, 64),
                skip_group_check=True,
            )
            nc.tensor.matmul(
                sum_part[0:64],
                ones64[:, :],
                pt[:, 0:QB],
                start=first,
                stop=last,
                tile_position=(0, 0),
                skip_group_check=True,
            )
            nc.tensor.matmul(
                sum_part[64:128],
                ones64[:, :],
                pt[:, QB : 2 * QB],
                start=first,
                stop=last,
                tile_position=(0, 64),
                skip_group_check=True,
            )

        # one flat chunk stream, software-pipelined globally: QK/exp of
        # chunk i+2 is emitted before AV/sums of chunk i, across (Q, pair)
        # boundaries, so the ScalarE exp stream never stalls
        chunks = [
            (qi, pair, kc)
            for qi in range(n_q)
            for pair in range(2)
            for kc in range(n_kc)
        ]
        nch = len(chunks)

        def qk(i):
            qi, pair, kc = chunks[i]
            return emit_qk_exp(pair, slice(qi * QB, (qi + 1) * QB), kc)

        y_t = y_d.rearrange("(t p) e -> p t e", p=128)
        n_qt = qlen // 128
        y_state = {}

        def emit_proj(qt_i):
            ps = slice(qt_i * 128, (qt_i + 1) * 128)
            y_ps = st_pool.tile([128, 256], F32, tag="st", name="y_ps")
            nc.tensor.matmul(
                y_ps[:], ctx_sb[0][:, ps], wt_sb[0][:], start=True, stop=False
            )
            nc.tensor.matmul(
                y_ps[:], ctx_sb[1][:, ps], wt_sb[1][:], start=False, stop=True
            )
            if qt_i % 2 == 0:
                y_state["sb"] = y_pool.tile([128, 512], F32, bufs=4, name="y_sb2")
            half = slice((qt_i % 2) * 256, (qt_i % 2) * 256 + 256)
            nc.vector.tensor_add(y_state["sb"][:, half], y_ps[:], bias_bc[:])
            if qt_i % 2 == 1 or qt_i == n_qt - 1:
                lo = (qt_i // 2) * 2
                hi = min(lo + 2, n_qt)
                eng = nc.sync if (qt_i // 2) % 2 == 0 else nc.scalar
                eng.dma_start(y_t[:, lo:hi, :], y_state["sb"][:, 0 : (hi - lo) * 256])

        proj_done = 0
        pts = {}
        pts[0] = qk(0)
        if nch > 1:
            pts[1] = qk(1)
        part = None
        for i, (qi, pair, kc) in enumerate(chunks):
            if i + 2 < nch:
                pts[i + 2] = qk(i + 2)
            if kc == 0:
                part = part_pool.tile([128, 2 * QB], F32, tag="part", name="part")
            emit_av_sums(pair, part[:, 0:QB], part[:, QB : 2 * QB], pts.pop(i), kc)
            # previous Q-block's projection rides in this block's PE slack
            if qi >= 1 and pair == 0 and kc in (4, 6, 8, 10):
                qt_i = (qi - 1) * 4 + (kc - 4) // 2
                if qt_i < n_qt:
                    emit_proj(qt_i)
                    proj_done = qt_i + 1
            if kc == n_kc - 1:
                # stage partials to SBUF (releases the PSUM banks), then
                # normalize off the critical path. On the very last pair
                # ScalarE is idle: split the copy across ACT+DVE to shorten
                # the tail's serial chain.
                qs = slice(qi * QB, (qi + 1) * QB)
                stage = ep_pool.tile([128, 2 * QB], F32, tag="stage")
                last_ep = qi == n_q - 1 and pair == 1
                if last_ep:
                    nc.scalar.copy(stage[:, 0:QB], part[:, 0:QB])
                    nc.vector.tensor_copy(stage[:, QB : 2 * QB], part[:, QB : 2 * QB])
                else:
                    nc.vector.tensor_copy(stage[:], part[:])
                recip = ep_pool.tile([128, QB], F32, tag="recip")
                nc.vector.reciprocal_approx_fast(recip[:], stage[:, QB : 2 * QB])
                nc.vector.tensor_mul(ctx_sb[pair][:, qs], stage[:, 0:QB], recip[:])

        # remaining projection tiles (last Q-block)
        for qt_i in range(proj_done, n_qt):
            emit_proj(qt_i)

    nc.compile()
    return nc


# revision 26
# speedup vs baseline: 1.0077x; 1.0075x over previous
"""Multi-head attention (B=4, N=4096, E=256, H=4) + output projection on
8 Trainium2 NeuronCores.

Sharding: data-parallel over (batch, query-half) -> 8 shards. Each core
computes full 4-head attention for one batch's 2048 queries against that
batch's full K/V, applies the output projection (+bias), and writes its
[2048, 256] f32 slice. No collectives needed; the host concatenates.

Per-core kernel (flash-attention style, S^T layout, bf16 compute):
  for each 512-query block, head-pair (2 heads), 128-key chunk:
    S^T_h0,S^T_h1 = row-tiled TensorE matmuls (K=64 contraction) -> PSUM
    P^T = exp(S^T/8)  -- one ScalarE ACTIVATE [128,1024] -> SBUF bf16
    out^T += col-tiled (V_h0|V_h1) matmuls     (PSUM accumulate)
    rowsums += col-tiled (ones|ones) matmuls   (64-replica trick)
  ctxT = out^T * reciprocal(rowsums)           (VectorE, off critical path)
  y = ctxT.T @ W_out.T + b_out                 (TensorE, K=1 bias matmul)

Host-side prep is layout only: transpose/cast shards to bf16 in the
layouts the TensorE wants (lhsT convention), plus the V swizzle.
"""

import os
import tempfile
from contextlib import ExitStack

import ml_dtypes
import numpy as np

import concourse.bass as bass
import concourse.tile as tile
from concourse import bacc, mybir
from concourse.bass_utils import run_bass_kernel_spmd

BF16 = mybir.dt.bfloat16
F32 = mybir.dt.float32

B, N, E = 4, 4096, 256
H, D = 4, 64
QLEN = N // 2
N_CORES = 8

LAST_EXEC_TIME_NS = None
_NC_CACHE = {}


def _build(qlen=QLEN, seq=N, n_cores=N_CORES):
    n_kc = seq // 128
    n_q = qlen // 512
    QB = 512

    nc = bacc.Bacc("TRN2", target_bir_lowering=False, debug=False, num_devices=n_cores)

    qt_d = nc.dram_tensor("qt", [2, 128, qlen], BF16, kind="ExternalInput").ap()
    kt_d = nc.dram_tensor("kt", [2, 128, seq], BF16, kind="ExternalInput").ap()
    v_d = nc.dram_tensor("v", [128, n_kc * 256], BF16, kind="ExternalInput").ap()
    wt_d = nc.dram_tensor("wt", [2, 128, 256], BF16, kind="ExternalInput").ap()
    bias_d = nc.dram_tensor("bias", [1, 256], BF16, kind="ExternalInput").ap()
    y_d = nc.dram_tensor("y", [qlen, 256], F32, kind="ExternalOutput").ap()

    with tile.TileContext(nc) as tc, ExitStack() as ctx:
        const = ctx.enter_context(tc.tile_pool(name="const", bufs=1))
        ep_pool = ctx.enter_context(tc.tile_pool(name="ep", bufs=2))
        y_pool = ctx.enter_context(tc.tile_pool(name="ysb", bufs=2))
        st_pool = ctx.enter_context(tc.tile_pool(name="st", bufs=3, space="PSUM"))
        part_pool = ctx.enter_context(tc.tile_pool(name="part", bufs=1, space="PSUM"))
        pt_pool = ctx.enter_context(tc.tile_pool(name="pt", bufs=8))

        qt_sb = [
            const.tile([128, qlen], BF16, tag=f"qt{p}", name=f"qt_sb{p}")
            for p in range(2)
        ]
        kt_sb = [
            const.tile([128, seq], BF16, tag=f"kt{p}", name=f"kt_sb{p}")
            for p in range(2)
        ]
        v_sb = const.tile([128, n_kc * 256], BF16, tag="v")
        wt_sb = [
            const.tile([128, 256], BF16, tag=f"wt{p}", name=f"wt_sb{p}")
            for p in range(2)
        ]
        bias_sb = const.tile([1, 256], BF16, tag="bias")
        ones64 = const.tile([128, 64], BF16, tag="ones64")
        ones1 = const.tile([1, 128], BF16, tag="ones1")
        zbias = const.tile([128, 1], F32, tag="zbias")
        ctx_sb = [
            const.tile([128, qlen], BF16, tag=f"ctx{p}", name=f"ctx_sb{p}")
            for p in range(2)
        ]

        # DMA issue costs ~650ns/inst on a HWDGE queue: split the loads
        # across BOTH rings (SP + ACT engines) and order by first use.
        # SP ring: pair01 q/k (first chunks first); ACT ring: v, pair23, w.
        qsp = min(512, qlen)
        ksp = min(1024, seq)
        vsp = min(2048, n_kc * 256)
        ksp0 = min(512, seq)
        nc.sync.dma_start(bias_sb[:], bias_d)
        nc.sync.dma_start(qt_sb[0][:, 0:qsp], qt_d[0][:, 0:qsp])
        nc.sync.dma_start(kt_sb[0][:, 0:ksp0], kt_d[0][:, 0:ksp0])
        if ksp0 < ksp:
            nc.sync.dma_start(kt_sb[0][:, ksp0:ksp], kt_d[0][:, ksp0:ksp])
        nc.sync.dma_start(v_sb[:, 0:vsp], v_d[:, 0:vsp])
        if qsp < qlen:
            nc.sync.dma_start(qt_sb[0][:, qsp:], qt_d[0][:, qsp:])
        if ksp < seq:
            nc.sync.dma_start(kt_sb[0][:, ksp:], kt_d[0][:, ksp:])
        if vsp < n_kc * 256:
            nc.sync.dma_start(v_sb[:, vsp:], v_d[:, vsp:])
        nc.sync.dma_start(qt_sb[1][:], qt_d[1])
        nc.sync.dma_start(kt_sb[1][:], kt_d[1])
        for p in range(2):
            nc.sync.dma_start(wt_sb[p][:], wt_d[p])
        bias_bc = const.tile([128, 256], F32, tag="bias_bc")
        # HAM warmup: ~6us of dependency-free matmuls so the PE clock-gate
        # is at 2.4 GHz when the first real QK lands (runs during the DMA
        # lead-in; uses the part slot, whose first real use is much later)
        dummy = const.tile([128, 512], BF16, tag="dummy")
        nc.vector.memset(dummy[:], 0.0)
        nc.vector.memset(ones64[:], 1.0)
        nc.vector.memset(ones1[:], 1.0)
        nc.vector.memset(zbias[:], 0.0)
        warm_ps = part_pool.tile([128, 2 * QB], F32, tag="part", name="warm_ps")
        # b_out broadcast to 128 partitions (projection epilogue adds);
        # computed during the DMA lead-in in the warmup's PSUM slot
        nc.tensor.matmul(
            warm_ps[:, QB : QB + 256], ones1[:, 0:128], bias_sb[:], start=True, stop=True
        )
        nc.vector.tensor_copy(bias_bc[:], warm_ps[:, QB : QB + 256])
        for _ in range(9):
            nc.tensor.matmul(
                warm_ps[0:64, 0:QB], ones64[:, :], dummy[:, :], start=True, stop=True
            )

        def emit_qk_exp(pair, qs, kc):
            ks = slice(kc * 128, (kc + 1) * 128)
            st = st_pool.tile([128, 2 * QB], F32, name="st")
            nc.tensor.matmul(
                st[:, 0:QB],
                kt_sb[pair][0:64, ks],
                qt_sb[pair][0:64, qs],
                start=True,
                stop=True,
                tile_position=(0, 0),
            )
            nc.tensor.matmul(
                st[:, QB : 2 * QB],
                kt_sb[pair][64:128, ks],
                qt_sb[pair][64:128, qs],
                start=True,
                stop=True,
                tile_position=(64, 0),
            )
            pt = pt_pool.tile([128, 2 * QB], BF16, name="pt")
            nc.scalar.activation(
                pt[:],
                st[:],
                mybir.ActivationFunctionType.Exp,
                bias=zbias[:, 0:1],
                scale=0.125,
            )
            return pt

        def emit_av_sums(pair, out_part, sum_part, pt, kc):
            first = kc == 0
            last = kc == n_kc - 1
            vcol = kc * 256 + pair * 128
            nc.tensor.matmul(
                out_part[0:64],
                v_sb[:, vcol : vcol + 64],
                pt[:, 0:QB],
                start=first,
                stop=last,
                tile_position=(0, 0),
                skip_group_check=True,
            )
            nc.tensor.matmul(
                out_part[64:128],
                v_sb[:, vcol + 64 : vcol + 128],
                pt[:, QB : 2 * QB],
                start=first,
                stop=last,
                tile_position=(0, 64),
                skip_group_check=True,
            )
            nc.tensor.matmul(
                sum_part[0:64],
                ones64[:, :],
                pt[:, 0:QB],
                start=first,
                stop=last,
                tile_position=(0, 0),
                skip_group_check=True,
            )
            nc.tensor.matmul(
                sum_part[64:128],
                ones64[:, :],
                pt[:, QB : 2 * QB],
                start=first,
                stop=last,
                tile_position=(0, 64),
                skip_group_check=True,
            )

        # one flat chunk stream, software-pipelined globally: QK/exp of
        # chunk i+2 is emitted before AV/sums of chunk i, across (Q, pair)
        # boundaries, so the ScalarE exp stream never stalls
        chunks = [
            (qi, pair, kc)
            for qi in range(n_q)
            for pair in range(2)
            for kc in range(n_kc)
        ]
        nch = len(chunks)

        def qk(i):
            qi, pair, kc = chunks[i]
            return emit_qk_exp(pair, slice(qi * QB, (qi + 1) * QB), kc)

        y_t = y_d.rearrange("(t p) e -> p t e", p=128)
        n_qt = qlen // 128
        y_state = {}

        def emit_proj(qt_i):
            ps = slice(qt_i * 128, (qt_i + 1) * 128)
            y_ps = st_pool.tile([128, 256], F32, tag="st", name="y_ps")
            nc.tensor.matmul(
                y_ps[:], ctx_sb[0][:, ps], wt_sb[0][:], start=True, stop=False
            )
            nc.tensor.matmul(
                y_ps[:], ctx_sb[1][:, ps], wt_sb[1][:], start=False, stop=True
            )
            if qt_i % 2 == 0:
                y_state["sb"] = y_pool.tile([128, 512], F32, bufs=4, name="y_sb2")
            half = slice((qt_i % 2) * 256, (qt_i % 2) * 256 + 256)
            nc.vector.tensor_add(y_state["sb"][:, half], y_ps[:], bias_bc[:])
            if qt_i % 2 == 1 or qt_i == n_qt - 1:
                lo = (qt_i // 2) * 2
                hi = min(lo + 2, n_qt)
                eng = nc.sync if (qt_i // 2) % 2 == 0 else nc.scalar
                eng.dma_start(y_t[:, lo:hi, :], y_state["sb"][:, 0 : (hi - lo) * 256])

        proj_done = 0
        pts = {}
        pts[0] = qk(0)
        if nch > 1:
            pts[1] = qk(1)
        part = None
        for i, (qi, pair, kc) in enumerate(chunks):
            if i + 2 < nch:
                pts[i + 2] = qk(i + 2)
            if kc == 0:
                part = part_pool.tile([128, 2 * QB], F32, tag="part", name="part")
            emit_av_sums(pair, part[:, 0:QB], part[:, QB : 2 * QB], pts.pop(i), kc)
            # previous Q-block's projection rides in this block's PE slack
            if qi >= 1 and pair == 0 and kc in (4, 6, 8, 10):
                qt_i = (qi - 1) * 4 + (kc - 4) // 2
                if qt_i < n_qt:
                    emit_proj(qt_i)
                    proj_done = qt_i + 1
            if kc == n_kc - 1:
                # stage partials to SBUF (releases the PSUM banks), then
                # normalize off the critical path. On the very last pair
                # ScalarE is idle: split the copy across ACT+DVE to shorten
                # the tail's serial chain.
                qs = slice(qi * QB, (qi + 1) * QB)
                stage = ep_pool.tile([128, 2 * QB], F32, tag="stage")
                last_ep = qi == n_q - 1 and pair == 1
                recip = ep_pool.tile([128, QB], F32, tag="recip")
                if last_ep:
                    # no successor needs the part banks: normalize straight
                    # from PSUM, skipping the staging copies (shorter chain)
                    nc.vector.reciprocal_approx_fast(recip[:], part[:, QB : 2 * QB])
                    nc.vector.tensor_mul(ctx_sb[pair][:, qs], part[:, 0:QB], recip[:])
                else:
                    nc.vector.tensor_copy(stage[:], part[:])
                    nc.vector.reciprocal_approx_fast(recip[:], stage[:, QB : 2 * QB])
                    nc.vector.tensor_mul(ctx_sb[pair][:, qs], stage[:, 0:QB], recip[:])

        # remaining projection tiles (last Q-block)
        for qt_i in range(proj_done, n_qt):
            emit_proj(qt_i)

    nc.compile()
    return nc
